# revision 11
# baseline (speedup 1.0000x reference)
"""DPConv (kernel=8, ext=4, stride=4) on 8 TRN2 NeuronCores — v12.

Math: with K = k + 2e = 16 and k = 8, DPConv collapses to
out = L @ img @ L.T per (n, c) image (L = exact 1-D operator, entries
n/16).  The column pass factors through natural pairs
P[s] = x[2s] + x[2s+1] (carrying the matmul's Lq = L/4 scaling):

    out[:, 4a+b] = P[:, 2a+b-2] + P[:, 2a+b]      (a = 1..30, b = 0..3)
    out[:, {0,1,126,127}] = T(x_col{0,127});  out[:, {2,3}] = 2 P[:, {0,1}]
    out[:, {124,125}] = 2 P[:, {63,63}] ... (see _stage edge block)

On-chip structure:
  * pairsum rides the matmul: host stages each image [evens64 | odds64];
    two accumulating matmuls (start/stop) give PSUM = Lq@e + Lq@o = P.
  * ALL edge columns come from one 512-col "E" matmul over 8 host-staged
    pre-scaled edge columns per image; ONE bulk strided DVE copy writes
    out cols {0..3,124..127} for all 64 images.
  * interior fold is ONE DVE tensor_add per group: inner dim b gives
    step-1 4-element bf16 runs starting at even element offsets, so the
    2x_1P packed mode still engages.
  * P evacuation PSUM->SBUF bf16 is ACT's only compute.

DMA plan: the load head is HBM-read-latency bound, so loads ride THREE
queues (SP ring / ACT ring / SWDGE) with >=2.3KB-per-partition
descriptors: sync [hdr+g0, g5+g6], scalar [g1+g2, g7+g8],
gpsimd [g3+g4].  Stores ride SWDGE (they wait only on their own
producers -> packets interleave with loads mid-kernel); the final small
store rides the then-idle ACT ring (0.6us HWDGE latency).
Group sizes (4,8x7,4): small first group starts the store stream early,
small last group shortens the loads-done -> last-byte drain.

Sharding: pure data parallel — core k takes batch element n = k.
"""

import ml_dtypes
import numpy as np

import concourse.bacc as bacc
import concourse.mybir as mybir
import concourse.tile as tile
from concourse import bass_utils
from concourse.ap import AP

N_CORES = 8
C_PER_CORE = 64
GROUPS = (4, 8, 8, 8, 8, 8, 8, 8, 4)
STARTS = (0, 4, 12, 20, 28, 36, 44, 52, 60)
# load chunks: (groups, engine) — engines: 0=sync, 1=scalar, 2=gpsimd
CHUNKS = (((0,), 0), ((1,), 1), ((3,), 0), ((2,), 1), ((5,), 0),
          ((4,), 1), ((7,), 0), ((6,), 1), ((8,), 1))
N_WARM = 8               # PE warmup matmuls (N=512) during the load head
LT0, E0, IMG0 = 0, 128, 640
XB_COLS = 8832
F32 = mybir.dt.float32
BF16 = mybir.dt.bfloat16
BF16_NP = ml_dtypes.bfloat16
assert sum(GROUPS) == C_PER_CORE


def _build_lq() -> np.ndarray:
    """The 1-D DPConv operator with the column-pass 1/4 folded in: L/4."""
    L = np.zeros((128, 128), np.float64)
    for w in range(128):
        i_lo = max(0, -((7 - w) // 4))      # ceil((w-7)/4)
        i_hi = min(30, w // 4)
        for i in (i_lo, i_hi):              # counted twice when equal
            L[w, min(127, max(0, 2 * w - 4 * i - 4))] += 0.25
            L[w, min(127, max(0, 2 * w - 4 * i - 3))] += 0.25
    return (L / 4.0).astype(np.float32)


_LQ_T = np.ascontiguousarray(_build_lq().T)          # lhsT layout [r, h]
_LQ_T_BF16 = _LQ_T.astype(BF16_NP)
assert np.all(_LQ_T_BF16.astype(np.float32) == _LQ_T)  # L exact in bf16


def _as_strided(base: AP, dims) -> AP:
    return AP(base.tensor, base.offset, dims)


def _flat(ap: AP, n: int) -> AP:
    pdim = list(ap.ap[0])
    return AP(ap.tensor, ap.offset, [pdim, [1, n]])


def _dpconv_tile(tc, o_d, xb_d):
    nc = tc.nc
    engs = [nc.sync, nc.scalar, nc.gpsimd]
    with tc.tile_pool(name="const", bufs=1) as cp, \
         tc.tile_pool(name="in", bufs=1) as inp, \
         tc.tile_pool(name="io", bufs=1) as iop, \
         tc.tile_pool(name="mid", bufs=4) as mp, \
         tc.tile_pool(name="ps", bufs=4, space="PSUM") as pp, \
         tc.tile_pool(name="psE", bufs=1, space="PSUM") as ppE:
        # ---- all load DMAs first, split across the two HWDGE rings ----
        img_tiles = {}
        with tc.high_priority():
            # chunk 0 = lt + E + g0 in ONE dma (2.3KB/partition descs)
            hdr = cp.tile([128, 640 + GROUPS[0] * 128], BF16)
            nc.sync.dma_start(out=hdr[:], in_=xb_d[:, 0:1152])
            img_tiles[0] = (hdr, 640)
            for gs, e in CHUNKS[1:]:
                n = sum(GROUPS[g] for g in gs)
                i0 = STARTS[gs[0]]
                ct = inp.tile([128, n, 128], BF16, tag=f"in{gs[0]}",
                              name=f"ct{gs[0]}")
                assert list(ct[:].ap[1])[0] == 128
                engs[e].dma_start(
                    out=_flat(ct[:], n * 128),
                    in_=xb_d[:, IMG0 + i0 * 128:IMG0 + (i0 + n) * 128])
                for g in gs:
                    img_tiles[g] = (ct, (STARTS[g] - i0) * 128)

            # ---- PE warmup: the HAM clock gate releases only after
            # ~3.4us of sustained PE activity; real matmuls start ~5us in
            # (after chunk0 lands), so burn the dead head on dummy
            # matmuls over a memset tile — every real matmul then runs
            # at 2.4 GHz instead of 1.2 ----
            wt = cp.tile([128, 512], BF16, tag="warm")
            nc.gpsimd.memset(wt[:], 0.0)
            wp = ppE.tile([128, 512], F32, tag="warmps")
            for _ in range(N_WARM):
                nc.tensor.matmul(wp[:], wt[:, 0:128], wt[:],
                                 start=True, stop=True)
        lt = hdr[:, 0:128]

        # edge matmul: psE[:, i, s] = final out col values {0..3,124..127}
        psE = ppE.tile([128, C_PER_CORE, 8], F32)
        assert list(psE[:].ap[1])[0] == 8
        nc.tensor.matmul(psE[:], lt, hdr[:, 128:640], start=True, stop=True)

        # one big output tile; stores slice it (strided-region deps)
        ot = iop.tile([128, C_PER_CORE, 128], BF16, tag="out")
        assert list(ot[:].ap[1])[0] == 128
        od0 = list(ot[:].ap[0])

        # ONE bulk edge copy for all 64 images (DVE, ready early)
        nc.vector.tensor_copy(
            out=_as_strided(ot[:, 0:1, 0:1],
                            [od0, [128, 64], [124, 2], [1, 4]]),
            in_=_as_strided(psE[:, 0:1, 0:1],
                            [list(psE[:].ap[0]), [8, 64], [4, 2], [1, 4]]))

        for g, (i0, G) in enumerate(zip(STARTS, GROUPS)):
            ct, cofs = img_tiles[g]

            def img_ap(lo, hi):
                off = ct[:].offset + cofs + lo
                return AP(ct[:].tensor, off,
                          [list(ct[:].ap[0]), [128, G], [1, hi - lo]])

            # pairsum-in-PSUM: P = Lq@evens + Lq@odds (accumulate)
            pt = pp.tile([128, 8, 64], F32, tag="P")
            assert list(pt[:].ap[1])[0] == 64
            po = pt[:, 0:G, :]
            nc.tensor.matmul(po, lt, img_ap(0, 64), start=True, stop=False)
            nc.tensor.matmul(po, lt, img_ap(64, 128), start=False, stop=True)

            # ACT: evacuate P to SBUF bf16
            ps = mp.tile([128, 8, 64], BF16, tag="P16")
            assert list(ps[:].ap[1])[0] == 64
            nc.scalar.copy(out=ps[:, 0:G, :], in_=pt[:, 0:G, :])

            pd = list(ps[:].ap[0])
            gdim = [64, G]
            ogdim = [128, G]

            # DVE interior fold, ONE op: out[4a+b] = P[2a+b-2] + P[2a+b],
            # a=1..30, b=0..3 — step-1 4-wide runs at even offsets
            nc.vector.tensor_add(
                out=_as_strided(ot[:, i0:i0 + 1, 4:5],
                                [od0, ogdim, [4, 30], [1, 4]]),
                in0=_as_strided(ps[:, 0:1, 0:1], [pd, gdim, [2, 30], [1, 4]]),
                in1=_as_strided(ps[:, 0:1, 2:3], [pd, gdim, [2, 30], [1, 4]]))

            # store: SWDGE for the body, ACT HWDGE ring for the final group
            st_eng = nc.scalar if g == len(GROUPS) - 1 else nc.gpsimd
            st_eng.dma_start(
                out=o_d[:, i0 * 128:(i0 + G) * 128],
                in_=_flat(ot[:, i0:i0 + G, :], G * 128))


_CACHE = {}


def _get_nc():
    if "nc" not in _CACHE:
        nc = bacc.Bacc("TRN2", target_bir_lowering=False, debug=False)
        xb_d = nc.dram_tensor("xb", (128, XB_COLS), BF16,
                              kind="ExternalInput").ap()
        o_d = nc.dram_tensor("o", (128, C_PER_CORE * 128), BF16,
                             kind="ExternalOutput").ap()
        with tile.TileContext(nc) as tc:
            _dpconv_tile(tc, o_d, xb_d)
        nc.compile()
        _CACHE["nc"] = nc
    return _CACHE["nc"]


def _stage(xk: np.ndarray) -> np.ndarray:
    """[C,H,W] f32 -> [128, XB_COLS] bf16: [lt | E | images as
    [evens|odds]], H-major so DMA reads are long contiguous runs."""
    t = xk.transpose(1, 0, 2)                      # [H, C, W]
    out = np.empty((128, XB_COLS), np.float32)
    out[:, 0:128] = _LQ_T
    E = np.empty((128, C_PER_CORE, 8), np.float32)
    E[:, :, 0] = E[:, :, 1] = 4.0 * t[:, :, 0]
    E[:, :, 2] = 2.0 * (t[:, :, 0] + t[:, :, 1])
    E[:, :, 3] = 2.0 * (t[:, :, 2] + t[:, :, 3])
    E[:, :, 4] = 2.0 * (t[:, :, 124] + t[:, :, 125])
    E[:, :, 5] = 2.0 * (t[:, :, 126] + t[:, :, 127])
    E[:, :, 6] = E[:, :, 7] = 4.0 * t[:, :, 127]
    out[:, E0:E0 + 512] = E.reshape(128, 512)
    img = np.concatenate([t[:, :, 0::2], t[:, :, 1::2]], axis=2)  # [H,C,128]
    out[:, IMG0:] = img.reshape(128, -1)
    return out.astype(BF16_NP)


def run(x: np.ndarray, **spmd_kwargs) -> bass_utils.BassKernelResults:
    """Shard x (8,64,128,128) across 8 cores and run the Bass kernel."""
    nc = _get_nc()
    in_maps = [{"xb": _stage(x[k])} for k in range(N_CORES)]
    return bass_utils.run_bass_kernel_spmd(
        nc, in_maps, core_ids=list(range(N_CORES)), **spmd_kwargs)


def kernel(x) -> np.ndarray:
    x = np.asarray(x, dtype=np.float32)
    assert x.shape == (N_CORES, C_PER_CORE, 128, 128), x.shape
    res = run(x)
    return np.stack(
        [res.results[k]["o"].reshape(128, C_PER_CORE, 128)
         .astype(np.float32).transpose(1, 0, 2)
         for k in range(N_CORES)],
        axis=0)


# revision 13
# speedup vs baseline: 1.0632x; 1.0632x over previous
"""DPConv (kernel=8, ext=4, stride=4) on 8 TRN2 NeuronCores — v12.

Math: with K = k + 2e = 16 and k = 8, DPConv collapses to
out = L @ img @ L.T per (n, c) image (L = exact 1-D operator, entries
n/16).  The column pass factors through natural pairs
P[s] = x[2s] + x[2s+1] (carrying the matmul's Lq = L/4 scaling):

    out[:, 4a+b] = P[:, 2a+b-2] + P[:, 2a+b]      (a = 1..30, b = 0..3)
    out[:, {0,1,126,127}] = T(x_col{0,127});  out[:, {2,3}] = 2 P[:, {0,1}]
    out[:, {124,125}] = 2 P[:, {63,63}] ... (see _stage edge block)

On-chip structure:
  * pairsum rides the matmul: host stages each image [evens64 | odds64];
    two accumulating matmuls (start/stop) give PSUM = Lq@e + Lq@o = P.
  * ALL edge columns come from one 512-col "E" matmul over 8 host-staged
    pre-scaled edge columns per image; ONE bulk strided DVE copy writes
    out cols {0..3,124..127} for all 64 images.
  * interior fold is ONE DVE tensor_add per group: inner dim b gives
    step-1 4-element bf16 runs starting at even element offsets, so the
    2x_1P packed mode still engages.
  * P evacuation PSUM->SBUF bf16 is ACT's only compute.

DMA plan: the load head is HBM-read-latency bound, so loads ride THREE
queues (SP ring / ACT ring / SWDGE) with >=2.3KB-per-partition
descriptors: sync [hdr+g0, g5+g6], scalar [g1+g2, g7+g8],
gpsimd [g3+g4].  Stores ride SWDGE (they wait only on their own
producers -> packets interleave with loads mid-kernel); the final small
store rides the then-idle ACT ring (0.6us HWDGE latency).
Group sizes (4,8x7,4): small first group starts the store stream early,
small last group shortens the loads-done -> last-byte drain.

Sharding: pure data parallel — core k takes batch element n = k.
"""

import ml_dtypes
import numpy as np

import concourse.bacc as bacc
import concourse.mybir as mybir
import concourse.tile as tile
from concourse import bass_utils
from concourse.ap import AP

N_CORES = 8
C_PER_CORE = 64
GROUPS = (4, 8, 8, 8, 8, 8, 8, 8, 4)
STARTS = (0, 4, 12, 20, 28, 36, 44, 52, 60)
# load chunks: (groups, engine) — engines: 0=sync, 1=scalar, 2=gpsimd
CHUNKS = (((0,), 0), ((1, 2), 1), ((3, 4), 0), ((5, 6), 1), ((7, 8), 0))
N_WARM = 4               # PE warmup matmuls (N=512) during the load head
LT0, E0, IMG0 = 0, 128, 640
XB_COLS = 8832
F32 = mybir.dt.float32
BF16 = mybir.dt.bfloat16
BF16_NP = ml_dtypes.bfloat16
assert sum(GROUPS) == C_PER_CORE


def _build_lq() -> np.ndarray:
    """The 1-D DPConv operator with the column-pass 1/4 folded in: L/4."""
    L = np.zeros((128, 128), np.float64)
    for w in range(128):
        i_lo = max(0, -((7 - w) // 4))      # ceil((w-7)/4)
        i_hi = min(30, w // 4)
        for i in (i_lo, i_hi):              # counted twice when equal
            L[w, min(127, max(0, 2 * w - 4 * i - 4))] += 0.25
            L[w, min(127, max(0, 2 * w - 4 * i - 3))] += 0.25
    return (L / 4.0).astype(np.float32)


_LQ_T = np.ascontiguousarray(_build_lq().T)          # lhsT layout [r, h]
_LQ_T_BF16 = _LQ_T.astype(BF16_NP)
assert np.all(_LQ_T_BF16.astype(np.float32) == _LQ_T)  # L exact in bf16


def _as_strided(base: AP, dims) -> AP:
    return AP(base.tensor, base.offset, dims)


def _flat(ap: AP, n: int) -> AP:
    pdim = list(ap.ap[0])
    return AP(ap.tensor, ap.offset, [pdim, [1, n]])


def _dpconv_tile(tc, o_d, xb_d):
    nc = tc.nc
    engs = [nc.sync, nc.scalar, nc.gpsimd]
    with tc.tile_pool(name="const", bufs=1) as cp, \
         tc.tile_pool(name="in", bufs=1) as inp, \
         tc.tile_pool(name="io", bufs=1) as iop, \
         tc.tile_pool(name="mid", bufs=4) as mp, \
         tc.tile_pool(name="ps", bufs=6, space="PSUM") as pp, \
         tc.tile_pool(name="psE", bufs=1, space="PSUM") as ppE:
        # ---- all load DMAs first, split across the two HWDGE rings ----
        img_tiles = {}
        with tc.high_priority():
            # chunk 0 = lt + E + g0 in ONE dma (2.3KB/partition descs)
            hdr = cp.tile([128, 640 + GROUPS[0] * 128], BF16)
            nc.sync.dma_start(out=hdr[:], in_=xb_d[:, 0:1152])
            img_tiles[0] = (hdr, 640)
            for gs, e in CHUNKS[1:]:
                n = sum(GROUPS[g] for g in gs)
                i0 = STARTS[gs[0]]
                ct = inp.tile([128, n, 128], BF16, tag=f"in{gs[0]}",
                              name=f"ct{gs[0]}")
                assert list(ct[:].ap[1])[0] == 128
                engs[e].dma_start(
                    out=_flat(ct[:], n * 128),
                    in_=xb_d[:, IMG0 + i0 * 128:IMG0 + (i0 + n) * 128])
                for g in gs:
                    img_tiles[g] = (ct, (STARTS[g] - i0) * 128)

            # ---- PE warmup: the HAM clock gate releases only after
            # ~3.4us of sustained PE activity; real matmuls start ~5us in
            # (after chunk0 lands), so burn the dead head on dummy
            # matmuls over a memset tile — every real matmul then runs
            # at 2.4 GHz instead of 1.2 ----
            wt = cp.tile([128, 512], BF16, tag="warm")
            nc.gpsimd.memset(wt[:], 0.0)
            wp = ppE.tile([128, 512], F32, tag="warmps")
            for _ in range(N_WARM):
                nc.tensor.matmul(wp[:], wt[:, 0:128], wt[:],
                                 start=True, stop=True)
        lt = hdr[:, 0:128]

        # edge matmul: psE[:, i, s] = final out col values {0..3,124..127}
        psE = ppE.tile([128, C_PER_CORE, 8], F32)
        assert list(psE[:].ap[1])[0] == 8
        nc.tensor.matmul(psE[:], lt, hdr[:, 128:640], start=True, stop=True)

        # one big output tile; stores slice it (strided-region deps)
        ot = iop.tile([128, C_PER_CORE, 128], BF16, tag="out")
        assert list(ot[:].ap[1])[0] == 128
        od0 = list(ot[:].ap[0])

        # ONE bulk edge copy for all 64 images (DVE, ready early)
        nc.vector.tensor_copy(
            out=_as_strided(ot[:, 0:1, 0:1],
                            [od0, [128, 64], [124, 2], [1, 4]]),
            in_=_as_strided(psE[:, 0:1, 0:1],
                            [list(psE[:].ap[0]), [8, 64], [4, 2], [1, 4]]))

        for g, (i0, G) in enumerate(zip(STARTS, GROUPS)):
            ct, cofs = img_tiles[g]

            def img_ap(lo, hi):
                off = ct[:].offset + cofs + lo
                return AP(ct[:].tensor, off,
                          [list(ct[:].ap[0]), [128, G], [1, hi - lo]])

            # pairsum-in-PSUM: P = Lq@evens + Lq@odds (accumulate)
            pt = pp.tile([128, 8, 64], F32, tag="P")
            assert list(pt[:].ap[1])[0] == 64
            po = pt[:, 0:G, :]
            nc.tensor.matmul(po, lt, img_ap(0, 64), start=True, stop=False)
            nc.tensor.matmul(po, lt, img_ap(64, 128), start=False, stop=True)

            # ACT: evacuate P to SBUF bf16
            ps = mp.tile([128, 8, 64], BF16, tag="P16")
            assert list(ps[:].ap[1])[0] == 64
            nc.scalar.copy(out=ps[:, 0:G, :], in_=pt[:, 0:G, :])

            pd = list(ps[:].ap[0])
            gdim = [64, G]
            ogdim = [128, G]

            # DVE interior fold, ONE op: out[4a+b] = P[2a+b-2] + P[2a+b],
            # a=1..30, b=0..3 — step-1 4-wide runs at even offsets
            nc.vector.tensor_add(
                out=_as_strided(ot[:, i0:i0 + 1, 4:5],
                                [od0, ogdim, [4, 30], [1, 4]]),
                in0=_as_strided(ps[:, 0:1, 0:1], [pd, gdim, [2, 30], [1, 4]]),
                in1=_as_strided(ps[:, 0:1, 2:3], [pd, gdim, [2, 30], [1, 4]]))

            # store: SWDGE for the body, ACT HWDGE ring for the final group
            st_eng = nc.scalar if g == len(GROUPS) - 1 else nc.gpsimd
            st_eng.dma_start(
                out=o_d[:, i0 * 128:(i0 + G) * 128],
                in_=_flat(ot[:, i0:i0 + G, :], G * 128))


_CACHE = {}


def _get_nc():
    if "nc" not in _CACHE:
        nc = bacc.Bacc("TRN2", target_bir_lowering=False, debug=False)
        xb_d = nc.dram_tensor("xb", (128, XB_COLS), BF16,
                              kind="ExternalInput").ap()
        o_d = nc.dram_tensor("o", (128, C_PER_CORE * 128), BF16,
                             kind="ExternalOutput").ap()
        with tile.TileContext(nc) as tc:
            _dpconv_tile(tc, o_d, xb_d)
        nc.compile()
        _CACHE["nc"] = nc
    return _CACHE["nc"]


def _stage(xk: np.ndarray) -> np.ndarray:
    """[C,H,W] f32 -> [128, XB_COLS] bf16: [lt | E | images as
    [evens|odds]], H-major so DMA reads are long contiguous runs."""
    t = xk.transpose(1, 0, 2)                      # [H, C, W]
    out = np.empty((128, XB_COLS), np.float32)
    out[:, 0:128] = _LQ_T
    E = np.empty((128, C_PER_CORE, 8), np.float32)
    E[:, :, 0] = E[:, :, 1] = 4.0 * t[:, :, 0]
    E[:, :, 2] = 2.0 * (t[:, :, 0] + t[:, :, 1])
    E[:, :, 3] = 2.0 * (t[:, :, 2] + t[:, :, 3])
    E[:, :, 4] = 2.0 * (t[:, :, 124] + t[:, :, 125])
    E[:, :, 5] = 2.0 * (t[:, :, 126] + t[:, :, 127])
    E[:, :, 6] = E[:, :, 7] = 4.0 * t[:, :, 127]
    out[:, E0:E0 + 512] = E.reshape(128, 512)
    img = np.concatenate([t[:, :, 0::2], t[:, :, 1::2]], axis=2)  # [H,C,128]
    out[:, IMG0:] = img.reshape(128, -1)
    return out.astype(BF16_NP)


def run(x: np.ndarray, **spmd_kwargs) -> bass_utils.BassKernelResults:
    """Shard x (8,64,128,128) across 8 cores and run the Bass kernel."""
    nc = _get_nc()
    in_maps = [{"xb": _stage(x[k])} for k in range(N_CORES)]
    return bass_utils.run_bass_kernel_spmd(
        nc, in_maps, core_ids=list(range(N_CORES)), **spmd_kwargs)


def kernel(x) -> np.ndarray:
    x = np.asarray(x, dtype=np.float32)
    assert x.shape == (N_CORES, C_PER_CORE, 128, 128), x.shape
    res = run(x)
    return np.stack(
        [res.results[k]["o"].reshape(128, C_PER_CORE, 128)
         .astype(np.float32).transpose(1, 0, 2)
         for k in range(N_CORES)],
        axis=0)
